# revision 14
# baseline (speedup 1.0000x reference)
"""Trainium2 Bass kernel for CapsuleLayer (dynamic routing, 3 iterations).

Math (per batch b, input capsule i):
  u_hat[b,i,n,d] = sum_k x[b,i,k] * W[i,n,d,k]          # [B, I, N, D]
  b0 = 0; c_r = softmax(b_r over n)
  out_r = relu(sum_n c_r * u_hat)                        # [B, I, D]
  b_{r+1} = b_r + sum_d out_r * u_hat                    # [B, I, N]
  return out_2

Sharding: input capsules I=2048 are sharded 8 ways (256 per core); routing is
per-(b,i) so no cross-core communication is needed, and each core streams only
its own W shard (W dominates memory traffic, so partitioning it beats
replicating it).

Per-core device program:
  - Host pre-transposes W to [i, k, (n,d)] and appends 32 mean-over-n columns
    (these produce the r=0 uniform-softmax weighted sum inside the matmul).
  - Main einsum: per group of 4 capsules, 4 concurrent 32x32 PE tiles
    (tile_position=(32r,32r)); stationary = x[:,i,:]^T, moving = W rhs.
    PSUM holds u_hat as [p=(i4,b), (n,d)], which is what routing wants.
  - Routing per 32-capsule chunk, vector-engine bound:
      * agreements: GPSIMD broadcasts the d-vector over n, then one fused
        custom DVE op (multiply + running cumsum) and a segment-end diff
        replace the separate multiply and reduce passes.
      * weighted sums: tensor_tensor multiply with broadcast AP + in-place
        binary-tree adds (contiguous; a strided tensor_reduce is ~1.6x
        slower).
      * softmax: max-stabilized, exp on the scalar engine, 1/Z folded into
        the relu-scale fixup (scalar_tensor_tensor).
"""

import re
import sys

if "/opt/trn_rl_repo" not in sys.path:
    sys.path.insert(0, "/opt/trn_rl_repo")

import numpy as np

import concourse.bacc as bacc
import concourse.bass as bass
import concourse.mybir as mybir
import concourse.tile as tile
from concourse import dve_ops
from concourse.bass_utils import run_bass_kernel_spmd
from concourse.dve_spec import AluOp as DveAluOp
from concourse.dve_spec import Spec, Src0, Src1, scan
from concourse.dve_table_gen import dve_ver_for

B, I, N, D = 32, 2048, 32, 32
NCORES = 8
IP = I // NCORES        # 256 capsules per core
NB = IP // 4            # 64 groups of 4 capsules
NCH = 8                 # routing chunks per core
BPC = NB // NCH         # 8 groups per routing chunk (32 capsules)
ND = N * D              # 1024 u_hat cols per capsule
FW = ND + D             # 1056 = u_hat cols + mean cols
F32 = mybir.dt.float32
AX = mybir.AxisListType.X
OP = mybir.AluOpType

_CACHE: dict = {}


def _register_mult_cumsum() -> "dve_ops.DveOp":
    """out[p, k] = cumsum over the free stream of in0*in1 (fp32).

    Segment sums are recovered by differencing segment-end elements; for our
    magnitudes (|cum| < ~100) the fp32 cancellation error is ~1e-5 absolute.
    """
    name = "MULT_CUMSUM_ANT"
    for op in dve_ops.OPS:
        if op.name == name:
            return op

    def _ref(in0, in1, c0, c1, c2):
        x = in0.astype(np.float32) * in1.astype(np.float32)
        f = x.reshape(x.shape[0], -1)
        return np.cumsum(f, axis=1).reshape(x.shape).astype(np.float32)

    op = dve_ops.DveOp(
        name,
        Spec(body=scan(DveAluOp.ADD, Src0 * Src1), reference=_ref),
        subdim=False,
        uops_sha={"v3": "b3fc3e78a862b7eb"},
    )
    dve_ops.OPS.append(op)
    dve_ops._SUB_OPCODE_FOR_NAME[name] = (
        max(dve_ops._SUB_OPCODE_FOR_NAME.values()) + 1
    )
    assert dve_ops._SUB_OPCODE_FOR_NAME[name] < 0x20
    ver = dve_ver_for("TRN2")
    try:
        op.compile(ver)
    except ValueError as e:  # sha drift (repo update): re-pin at runtime
        m = re.search(r'\]="([0-9a-f]+)"', str(e))
        if not m:
            raise
        op.uops_sha[ver] = m.group(1)
        dve_ops._COMPILE_CACHE.pop((name, ver), None)
        op.compile(ver)
    return op


def _build_bass() -> bass.Bass:
    mult_cumsum = _register_mult_cumsum()
    nc = bacc.Bacc(None, target_bir_lowering=False)
    w_in = nc.dram_tensor(
        "w_in", [NB // 2, 128, 2 * FW], F32, kind="ExternalInput"
    )
    x_in = nc.dram_tensor("x_in", [128, NB, B], F32, kind="ExternalInput")
    out_d = nc.dram_tensor("out", [128, NB, D], F32, kind="ExternalOutput")

    with tile.TileContext(nc) as tc:
        with (
            tc.tile_pool(name="xp", bufs=1) as xp,
            tc.tile_pool(name="wp", bufs=4) as wp,
            tc.tile_pool(name="up", bufs=2) as up,
            tc.tile_pool(name="tp", bufs=1) as tp,
            tc.tile_pool(name="ep", bufs=1) as ep,
            tc.tile_pool(name="sp", bufs=1) as sp,
            tc.tile_pool(name="ps", bufs=2, space="PSUM") as ps,
        ):
            x_t = xp.tile([128, NB * B], F32)
            nc.sync.dma_start(
                x_t[:].rearrange("p (g b) -> p g b", g=NB), x_in[:]
            )

            for ch in range(NCH):
                u_t = up.tile([128, BPC * ND], F32, tag="u")
                mn_t = up.tile([128, BPC * D], F32, tag="mn")
                for gp in range(BPC // 2):
                    w_t = wp.tile([128, 2 * FW], F32, tag="w")
                    nc.sync.dma_start(w_t[:], w_in[ch * (BPC // 2) + gp])
                    for gj in range(2):
                        gi = gp * 2 + gj
                        g = ch * BPC + gi
                        wofs = gj * FW
                        p_t = ps.tile([128, FW], F32, tag="ps")
                        for r in range(4):
                            rows = slice(32 * r, 32 * r + 32)
                            lhsT = x_t[rows, g * B : (g + 1) * B]
                            tp_pos = (32 * r, 32 * r)
                            nc.tensor.matmul(
                                p_t[rows, 0:512],
                                lhsT, w_t[rows, wofs : wofs + 512],
                                tile_position=tp_pos,
                            )
                            nc.tensor.matmul(
                                p_t[rows, 512:1024],
                                lhsT, w_t[rows, wofs + 512 : wofs + 1024],
                                tile_position=tp_pos,
                            )
                            nc.tensor.matmul(
                                p_t[rows, 1024:1056],
                                lhsT, w_t[rows, wofs + 1024 : wofs + 1056],
                                tile_position=tp_pos,
                            )
                        # drain PSUM -> SBUF on scalar engine (closest to
                        # PSUM); u_hat and mean cols land in separate tiles
                        # so u_t stays (g,n,d)-contiguous for the fused ops
                        nc.scalar.copy(
                            u_t[:, gi * ND : (gi + 1) * ND], p_t[:, 0:ND]
                        )
                        nc.scalar.copy(
                            mn_t[:, gi * D : (gi + 1) * D], p_t[:, ND:FW]
                        )

                # ---- routing for this chunk (32 capsules, all 32 b) ----
                u3 = u_t[:].rearrange("p (s d) -> p s d", d=D)  # [p,(g n),d]
                u4 = u_t[:].rearrange("p (g n d) -> p g n d", g=BPC, n=N)
                mean_v = mn_t[:].rearrange("p (g d) -> p g d", g=BPC)
                GD, GN = BPC * D, BPC * N
                SEG = BPC * N            # segments per agreement pass
                shp4 = (128, BPC, N, D)

                # agreement 1: a1[g,n] = sum_d u * relu(mean)
                # relu+broadcast(n) of the mean on GPSIMD (idle engine)
                ex0 = ep.tile([128, BPC * ND], F32, tag="exp")
                ex0v = ex0[:].rearrange("p (g n d) -> p g n d", g=BPC, n=N)
                nc.gpsimd.tensor_scalar_max(
                    ex0v, mean_v.unsqueeze(2).broadcast_to(shp4), 0.0
                )
                cum = tp.tile([128, BPC * ND], F32, tag="tmp")
                cum3 = cum[:].rearrange("p (s d) -> p s d", d=D)
                nc.vector._custom_dve(
                    mult_cumsum, out=cum3, in0=u3,
                    in1=ex0[:].rearrange("p (s d) -> p s d", d=D),
                )
                ends = sp.tile([128, SEG + 1], F32, tag="ends")
                nc.vector.memset(ends[:, 0:1], 0.0)
                nc.vector.tensor_copy(ends[:, 1 : SEG + 1], cum3[:, :, D - 1])
                a1 = sp.tile([128, GN], F32, tag="a1")
                a1v = a1[:].rearrange("p (g n) -> p g n", g=BPC)
                nc.vector.tensor_sub(
                    a1[:], ends[:, 1 : SEG + 1], ends[:, 0:SEG]
                )

                # softmax 1 (max-stabilized; 1/Z folded into out1)
                m1 = sp.tile([128, BPC], F32, tag="m1")
                nc.vector.reduce_max(m1[:], a1v, axis=AX)
                s1 = sp.tile([128, GN], F32, tag="s1")
                s1v = s1[:].rearrange("p (g n) -> p g n", g=BPC)
                nc.vector.tensor_sub(
                    s1v, a1v, m1[:].unsqueeze(2).broadcast_to((128, BPC, N))
                )
                e1 = sp.tile([128, GN], F32, tag="e1")
                nc.scalar.activation(
                    e1[:], s1[:], mybir.ActivationFunctionType.Exp
                )
                e1v = e1[:].rearrange("p (g n) -> p g n", g=BPC)
                z1 = sp.tile([128, BPC], F32, tag="z1")
                nc.vector.reduce_sum(z1[:], e1v, axis=AX)
                r1 = sp.tile([128, BPC], F32, tag="r1")
                nc.vector.reciprocal(r1[:], z1[:])

                # weighted sum 1: out1 = relu(sum_n e1 * u) * r1
                t2 = tp.tile([128, BPC * ND], F32, tag="tmp")
                t2v = t2[:].rearrange("p (g n d) -> p g n d", g=BPC, n=N)
                nc.vector.tensor_mul(
                    t2v, u4, e1v.unsqueeze(3).broadcast_to(shp4)
                )
                half = N
                while half > 1:
                    half //= 2
                    nc.vector.tensor_add(
                        t2v[:, :, 0:half, :],
                        t2v[:, :, 0:half, :],
                        t2v[:, :, half : 2 * half, :],
                    )
                w1v = t2v[:, :, 0, :]
                out1 = sp.tile([128, GD], F32, tag="out1")
                o1v = out1[:].rearrange("p (g d) -> p g d", g=BPC)
                nc.vector.scalar_tensor_tensor(
                    o1v, w1v, 0.0,
                    r1[:].unsqueeze(2).broadcast_to((128, BPC, D)),
                    op0=OP.max, op1=OP.mult,
                )

                # agreement 2: b2 = a1 + sum_d u * out1
                ex1 = ep.tile([128, BPC * ND], F32, tag="exp")
                ex1v = ex1[:].rearrange("p (g n d) -> p g n d", g=BPC, n=N)
                nc.gpsimd.tensor_copy(
                    ex1v, o1v.unsqueeze(2).broadcast_to(shp4)
                )
                cum2 = tp.tile([128, BPC * ND], F32, tag="tmp")
                cum23 = cum2[:].rearrange("p (s d) -> p s d", d=D)
                nc.vector._custom_dve(
                    mult_cumsum, out=cum23, in0=u3,
                    in1=ex1[:].rearrange("p (s d) -> p s d", d=D),
                )
                ends2 = sp.tile([128, SEG + 1], F32, tag="ends")
                nc.vector.memset(ends2[:, 0:1], 0.0)
                nc.vector.tensor_copy(
                    ends2[:, 1 : SEG + 1], cum23[:, :, D - 1]
                )
                b2 = sp.tile([128, GN], F32, tag="b2")
                b2v = b2[:].rearrange("p (g n) -> p g n", g=BPC)
                nc.vector.tensor_sub(
                    b2[:], ends2[:, 1 : SEG + 1], ends2[:, 0:SEG]
                )
                nc.vector.tensor_add(b2[:], b2[:], a1[:])

                # softmax 2 (max-stabilized)
                m2 = sp.tile([128, BPC], F32, tag="m2")
                nc.vector.reduce_max(m2[:], b2v, axis=AX)
                s2 = sp.tile([128, GN], F32, tag="s2")
                s2v = s2[:].rearrange("p (g n) -> p g n", g=BPC)
                nc.vector.tensor_sub(
                    s2v, b2v, m2[:].unsqueeze(2).broadcast_to((128, BPC, N))
                )
                e2 = sp.tile([128, GN], F32, tag="e2")
                nc.scalar.activation(
                    e2[:], s2[:], mybir.ActivationFunctionType.Exp
                )
                e2v = e2[:].rearrange("p (g n) -> p g n", g=BPC)
                z2 = sp.tile([128, BPC], F32, tag="z2")
                nc.vector.reduce_sum(z2[:], e2v, axis=AX)
                r2 = sp.tile([128, BPC], F32, tag="r2")
                nc.vector.reciprocal(r2[:], z2[:])

                # weighted sum 2 -> final out
                t4 = tp.tile([128, BPC * ND], F32, tag="tmp")
                t4v = t4[:].rearrange("p (g n d) -> p g n d", g=BPC, n=N)
                nc.vector.tensor_mul(
                    t4v, u4, e2v.unsqueeze(3).broadcast_to(shp4)
                )
                half = N
                while half > 1:
                    half //= 2
                    nc.vector.tensor_add(
                        t4v[:, :, 0:half, :],
                        t4v[:, :, 0:half, :],
                        t4v[:, :, half : 2 * half, :],
                    )
                w2v = t4v[:, :, 0, :]
                fin = sp.tile([128, GD], F32, tag="fin")
                finv = fin[:].rearrange("p (g d) -> p g d", g=BPC)
                nc.vector.scalar_tensor_tensor(
                    finv, w2v, 0.0,
                    r2[:].unsqueeze(2).broadcast_to((128, BPC, D)),
                    op0=OP.max, op1=OP.mult,
                )
                nc.sync.dma_start(
                    out_d[:, ch * BPC : (ch + 1) * BPC, :], finv
                )
    nc.compile()
    return nc


def _get_nc() -> bass.Bass:
    if "nc" not in _CACHE:
        _CACHE["nc"] = _build_bass()
    return _CACHE["nc"]


def _prep_inputs(x: np.ndarray, W: np.ndarray) -> list[dict]:
    x = np.asarray(x, dtype=np.float32)
    W = np.asarray(W, dtype=np.float32)
    # W[i, n, d, k] -> [i, k, (n d)] plus mean-over-n columns [i, k, d]
    wt = np.ascontiguousarray(W.transpose(0, 3, 1, 2)).reshape(I, D, N * D)
    wm = np.ascontiguousarray(W.mean(axis=1).transpose(0, 2, 1)).astype(
        np.float32
    )
    wfull = np.concatenate([wt, wm], axis=2)  # [I, 32, 1056]
    # x[b, i, k] -> [i, k, b]
    xt = np.ascontiguousarray(x.transpose(1, 2, 0))  # [I, 32, B]

    in_maps = []
    for c in range(NCORES):
        wc = (
            wfull[c * IP : (c + 1) * IP]
            .reshape(NB // 2, 2, 128, FW)
            .transpose(0, 2, 1, 3)
            .reshape(NB // 2, 128, 2 * FW)
        )
        xc = (
            xt[c * IP : (c + 1) * IP]
            .reshape(NB, 4, D, B)
            .transpose(1, 2, 0, 3)
            .reshape(128, NB, B)
        )
        in_maps.append(
            {
                "w_in": np.ascontiguousarray(wc),
                "x_in": np.ascontiguousarray(xc),
            }
        )
    return in_maps


def _assemble_out(results: list[dict]) -> np.ndarray:
    parts = []
    for c in range(NCORES):
        o = results[c]["out"]  # [128, NB, D] with p = i4*32 + b
        o = o.reshape(4, 32, NB, D).transpose(1, 2, 0, 3).reshape(B, IP, D)
        parts.append(o)
    return np.ascontiguousarray(np.concatenate(parts, axis=1))


def _run(x: np.ndarray, W: np.ndarray, **kwargs):
    nc = _get_nc()
    in_maps = _prep_inputs(x, W)
    res = run_bass_kernel_spmd(
        nc, in_maps, core_ids=list(range(NCORES)), **kwargs
    )
    return _assemble_out(res.results), res


def kernel(x: np.ndarray, W: np.ndarray) -> np.ndarray:
    out, _ = _run(x, W)
    return out


# revision 16
# speedup vs baseline: 2.6073x; 2.6073x over previous
"""Trainium2 Bass kernel for CapsuleLayer (dynamic routing, 3 iterations).

Math (per batch b, input capsule i):
  u_hat[b,i,n,d] = sum_k x[b,i,k] * W[i,n,d,k]          # [B, I, N, D]
  b0 = 0; c_r = softmax(b_r over n)
  out_r = relu(sum_n c_r * u_hat)                        # [B, I, D]
  b_{r+1} = b_r + sum_d out_r * u_hat                    # [B, I, N]
  return out_2

Sharding: input capsules I=2048 are sharded 8 ways (256 per core); routing is
per-(b,i) so no cross-core communication is needed, and each core streams only
its own W shard (W dominates memory traffic, so partitioning it beats
replicating it).

Per-core device program:
  - Host pre-transposes W to [i, k, (n,d)] and appends 32 mean-over-n columns
    (these produce the r=0 uniform-softmax weighted sum inside the matmul).
  - Main einsum: per group of 4 capsules, 4 concurrent 32x32 PE tiles
    (tile_position=(32r,32r)); stationary = x[:,i,:]^T, moving = W rhs.
    PSUM holds u_hat as [p=(i4,b), (n,d)], which is what routing wants.
  - Routing per 32-capsule chunk, vector-engine bound:
      * agreements: GPSIMD broadcasts the d-vector over n, then one fused
        custom DVE op (multiply + running cumsum) and a segment-end diff
        replace the separate multiply and reduce passes.
      * weighted sums: tensor_tensor multiply with broadcast AP + in-place
        binary-tree adds (contiguous; a strided tensor_reduce is ~1.6x
        slower).
      * softmax: max-stabilized, exp on the scalar engine, 1/Z folded into
        the relu-scale fixup (scalar_tensor_tensor).
"""

import re
import sys

if "/opt/trn_rl_repo" not in sys.path:
    sys.path.insert(0, "/opt/trn_rl_repo")

import numpy as np

import concourse.bacc as bacc
import concourse.bass as bass
import concourse.mybir as mybir
import concourse.tile as tile
from concourse import dve_ops
from concourse.bass_utils import run_bass_kernel_spmd
from concourse.dve_spec import AluOp as DveAluOp
from concourse.dve_spec import Spec, Src0, Src1, scan
from concourse.dve_table_gen import dve_ver_for

B, I, N, D = 32, 2048, 32, 32
NCORES = 8
IP = I // NCORES        # 256 capsules per core
NB = IP // 4            # 64 groups of 4 capsules
NCH = 8                 # routing chunks per core
BPC = NB // NCH         # 8 groups per routing chunk (32 capsules)
ND = N * D              # 1024 u_hat cols per capsule
FW = ND + D             # 1056 = u_hat cols + mean cols
F32 = mybir.dt.float32
AX = mybir.AxisListType.X
OP = mybir.AluOpType

_CACHE: dict = {}


def _register_mult_cumsum() -> "dve_ops.DveOp":
    """out[p, k] = cumsum over the free stream of in0*in1 (fp32).

    Segment sums are recovered by differencing segment-end elements; for our
    magnitudes (|cum| < ~100) the fp32 cancellation error is ~1e-5 absolute.
    """
    name = "MULT_CUMSUM_ANT"
    for op in dve_ops.OPS:
        if op.name == name:
            return op

    def _ref(in0, in1, c0, c1, c2):
        x = in0.astype(np.float32) * in1.astype(np.float32)
        f = x.reshape(x.shape[0], -1)
        return np.cumsum(f, axis=1).reshape(x.shape).astype(np.float32)

    op = dve_ops.DveOp(
        name,
        Spec(body=scan(DveAluOp.ADD, Src0 * Src1), reference=_ref),
        subdim=False,
        uops_sha={"v3": "b3fc3e78a862b7eb"},
    )
    dve_ops.OPS.append(op)
    dve_ops._SUB_OPCODE_FOR_NAME[name] = (
        max(dve_ops._SUB_OPCODE_FOR_NAME.values()) + 1
    )
    assert dve_ops._SUB_OPCODE_FOR_NAME[name] < 0x20
    ver = dve_ver_for("TRN2")
    try:
        op.compile(ver)
    except ValueError as e:  # sha drift (repo update): re-pin at runtime
        m = re.search(r'\]="([0-9a-f]+)"', str(e))
        if not m:
            raise
        op.uops_sha[ver] = m.group(1)
        dve_ops._COMPILE_CACHE.pop((name, ver), None)
        op.compile(ver)
    return op


def _build_bass() -> bass.Bass:
    mult_cumsum = _register_mult_cumsum()
    nc = bacc.Bacc(None, target_bir_lowering=False)
    w_in = nc.dram_tensor(
        "w_in", [NB // 2, 128, 2 * FW], F32, kind="ExternalInput"
    )
    x_in = nc.dram_tensor("x_in", [128, NB, B], F32, kind="ExternalInput")
    out_d = nc.dram_tensor("out", [128, NB, D], F32, kind="ExternalOutput")

    with tile.TileContext(nc) as tc:
        with (
            tc.tile_pool(name="xp", bufs=1) as xp,
            tc.tile_pool(name="wp", bufs=4) as wp,
            tc.tile_pool(name="up", bufs=2) as up,
            tc.tile_pool(name="tp", bufs=1) as tp,
            tc.tile_pool(name="ep", bufs=1) as ep,
            tc.tile_pool(name="sp", bufs=1) as sp,
            tc.tile_pool(name="ps", bufs=2, space="PSUM") as ps,
        ):
            x_t = xp.tile([128, NB * B], F32)
            nc.sync.dma_start(
                x_t[:].rearrange("p (g b) -> p g b", g=NB), x_in[:]
            )

            for ch in range(NCH):
                u_t = up.tile([128, BPC * ND], F32, tag="u")
                mn_t = up.tile([128, BPC * D], F32, tag="mn")
                for gp in range(BPC // 2):
                    w_t = wp.tile([128, 2 * FW], F32, tag="w")
                    nc.sync.dma_start(w_t[:], w_in[ch * (BPC // 2) + gp])
                    for gj in range(2):
                        gi = gp * 2 + gj
                        g = ch * BPC + gi
                        wofs = gj * FW
                        p_t = ps.tile([128, FW], F32, tag="ps")
                        for r in range(4):
                            rows = slice(32 * r, 32 * r + 32)
                            lhsT = x_t[rows, g * B : (g + 1) * B]
                            tp_pos = (32 * r, 32 * r)
                            nc.tensor.matmul(
                                p_t[rows, 0:512],
                                lhsT, w_t[rows, wofs : wofs + 512],
                                tile_position=tp_pos,
                            )
                            nc.tensor.matmul(
                                p_t[rows, 512:1024],
                                lhsT, w_t[rows, wofs + 512 : wofs + 1024],
                                tile_position=tp_pos,
                            )
                            nc.tensor.matmul(
                                p_t[rows, 1024:1056],
                                lhsT, w_t[rows, wofs + 1024 : wofs + 1056],
                                tile_position=tp_pos,
                            )
                        # drain PSUM -> SBUF on scalar engine (closest to
                        # PSUM); u_hat and mean cols land in separate tiles
                        # so u_t stays (g,n,d)-contiguous for the fused ops
                        nc.scalar.copy(
                            u_t[:, gi * ND : (gi + 1) * ND], p_t[:, 0:ND]
                        )
                        nc.scalar.copy(
                            mn_t[:, gi * D : (gi + 1) * D], p_t[:, ND:FW]
                        )

                # ---- routing for this chunk (32 capsules, all 32 b) ----
                u3 = u_t[:].rearrange("p (s d) -> p s d", d=D)  # [p,(g n),d]
                u4 = u_t[:].rearrange("p (g n d) -> p g n d", g=BPC, n=N)
                mean_v = mn_t[:].rearrange("p (g d) -> p g d", g=BPC)
                GD, GN = BPC * D, BPC * N
                SEG = BPC * N            # segments per agreement pass
                shp4 = (128, BPC, N, D)

                # agreement 1: a1[g,n] = sum_d u * relu(mean)
                # relu+broadcast(n) of the mean on GPSIMD (idle engine)
                ex0 = ep.tile([128, BPC * ND], F32, tag="exp")
                ex0v = ex0[:].rearrange("p (g n d) -> p g n d", g=BPC, n=N)
                nc.scalar.activation(
                    ex0v, mean_v.unsqueeze(2).broadcast_to(shp4),
                    mybir.ActivationFunctionType.Relu,
                )
                cum = tp.tile([128, BPC * ND], F32, tag="tmp")
                cum3 = cum[:].rearrange("p (s d) -> p s d", d=D)
                nc.vector._custom_dve(
                    mult_cumsum, out=cum3, in0=u3,
                    in1=ex0[:].rearrange("p (s d) -> p s d", d=D),
                )
                ends = sp.tile([128, SEG + 1], F32, tag="ends")
                nc.vector.memset(ends[:, 0:1], 0.0)
                nc.vector.tensor_copy(ends[:, 1 : SEG + 1], cum3[:, :, D - 1])
                a1 = sp.tile([128, GN], F32, tag="a1")
                a1v = a1[:].rearrange("p (g n) -> p g n", g=BPC)
                nc.vector.tensor_sub(
                    a1[:], ends[:, 1 : SEG + 1], ends[:, 0:SEG]
                )

                # softmax 1 (max-stabilized; 1/Z folded into out1)
                m1 = sp.tile([128, BPC], F32, tag="m1")
                nc.vector.reduce_max(m1[:], a1v, axis=AX)
                s1 = sp.tile([128, GN], F32, tag="s1")
                s1v = s1[:].rearrange("p (g n) -> p g n", g=BPC)
                nc.vector.tensor_sub(
                    s1v, a1v, m1[:].unsqueeze(2).broadcast_to((128, BPC, N))
                )
                e1 = sp.tile([128, GN], F32, tag="e1")
                nc.scalar.activation(
                    e1[:], s1[:], mybir.ActivationFunctionType.Exp
                )
                e1v = e1[:].rearrange("p (g n) -> p g n", g=BPC)
                z1 = sp.tile([128, BPC], F32, tag="z1")
                nc.vector.reduce_sum(z1[:], e1v, axis=AX)
                r1 = sp.tile([128, BPC], F32, tag="r1")
                nc.vector.reciprocal(r1[:], z1[:])

                # weighted sum 1: out1 = relu(sum_n e1 * u) * r1
                t2 = tp.tile([128, BPC * ND], F32, tag="tmp")
                t2v = t2[:].rearrange("p (g n d) -> p g n d", g=BPC, n=N)
                nc.vector.tensor_mul(
                    t2v, u4, e1v.unsqueeze(3).broadcast_to(shp4)
                )
                half = N
                while half > 1:
                    half //= 2
                    nc.vector.tensor_add(
                        t2v[:, :, 0:half, :],
                        t2v[:, :, 0:half, :],
                        t2v[:, :, half : 2 * half, :],
                    )
                w1v = t2v[:, :, 0, :]
                out1 = sp.tile([128, GD], F32, tag="out1")
                o1v = out1[:].rearrange("p (g d) -> p g d", g=BPC)
                nc.vector.scalar_tensor_tensor(
                    o1v, w1v, 0.0,
                    r1[:].unsqueeze(2).broadcast_to((128, BPC, D)),
                    op0=OP.max, op1=OP.mult,
                )

                # agreement 2: b2 = a1 + sum_d u * out1
                ex1 = ep.tile([128, BPC * ND], F32, tag="exp")
                ex1v = ex1[:].rearrange("p (g n d) -> p g n d", g=BPC, n=N)
                nc.scalar.copy(
                    ex1v, o1v.unsqueeze(2).broadcast_to(shp4)
                )
                cum2 = tp.tile([128, BPC * ND], F32, tag="tmp")
                cum23 = cum2[:].rearrange("p (s d) -> p s d", d=D)
                nc.vector._custom_dve(
                    mult_cumsum, out=cum23, in0=u3,
                    in1=ex1[:].rearrange("p (s d) -> p s d", d=D),
                )
                ends2 = sp.tile([128, SEG + 1], F32, tag="ends")
                nc.vector.memset(ends2[:, 0:1], 0.0)
                nc.vector.tensor_copy(
                    ends2[:, 1 : SEG + 1], cum23[:, :, D - 1]
                )
                b2 = sp.tile([128, GN], F32, tag="b2")
                b2v = b2[:].rearrange("p (g n) -> p g n", g=BPC)
                nc.vector.tensor_sub(
                    b2[:], ends2[:, 1 : SEG + 1], ends2[:, 0:SEG]
                )
                nc.vector.tensor_add(b2[:], b2[:], a1[:])

                # softmax 2 (max-stabilized)
                m2 = sp.tile([128, BPC], F32, tag="m2")
                nc.vector.reduce_max(m2[:], b2v, axis=AX)
                s2 = sp.tile([128, GN], F32, tag="s2")
                s2v = s2[:].rearrange("p (g n) -> p g n", g=BPC)
                nc.vector.tensor_sub(
                    s2v, b2v, m2[:].unsqueeze(2).broadcast_to((128, BPC, N))
                )
                e2 = sp.tile([128, GN], F32, tag="e2")
                nc.scalar.activation(
                    e2[:], s2[:], mybir.ActivationFunctionType.Exp
                )
                e2v = e2[:].rearrange("p (g n) -> p g n", g=BPC)
                z2 = sp.tile([128, BPC], F32, tag="z2")
                nc.vector.reduce_sum(z2[:], e2v, axis=AX)
                r2 = sp.tile([128, BPC], F32, tag="r2")
                nc.vector.reciprocal(r2[:], z2[:])

                # weighted sum 2 -> final out
                t4 = tp.tile([128, BPC * ND], F32, tag="tmp")
                t4v = t4[:].rearrange("p (g n d) -> p g n d", g=BPC, n=N)
                nc.vector.tensor_mul(
                    t4v, u4, e2v.unsqueeze(3).broadcast_to(shp4)
                )
                half = N
                while half > 1:
                    half //= 2
                    nc.vector.tensor_add(
                        t4v[:, :, 0:half, :],
                        t4v[:, :, 0:half, :],
                        t4v[:, :, half : 2 * half, :],
                    )
                w2v = t4v[:, :, 0, :]
                fin = sp.tile([128, GD], F32, tag="fin")
                finv = fin[:].rearrange("p (g d) -> p g d", g=BPC)
                nc.vector.scalar_tensor_tensor(
                    finv, w2v, 0.0,
                    r2[:].unsqueeze(2).broadcast_to((128, BPC, D)),
                    op0=OP.max, op1=OP.mult,
                )
                nc.sync.dma_start(
                    out_d[:, ch * BPC : (ch + 1) * BPC, :], finv
                )
    nc.compile()
    return nc


def _get_nc() -> bass.Bass:
    if "nc" not in _CACHE:
        _CACHE["nc"] = _build_bass()
    return _CACHE["nc"]


def _prep_inputs(x: np.ndarray, W: np.ndarray) -> list[dict]:
    x = np.asarray(x, dtype=np.float32)
    W = np.asarray(W, dtype=np.float32)
    # W[i, n, d, k] -> [i, k, (n d)] plus mean-over-n columns [i, k, d]
    wt = np.ascontiguousarray(W.transpose(0, 3, 1, 2)).reshape(I, D, N * D)
    wm = np.ascontiguousarray(W.mean(axis=1).transpose(0, 2, 1)).astype(
        np.float32
    )
    wfull = np.concatenate([wt, wm], axis=2)  # [I, 32, 1056]
    # x[b, i, k] -> [i, k, b]
    xt = np.ascontiguousarray(x.transpose(1, 2, 0))  # [I, 32, B]

    in_maps = []
    for c in range(NCORES):
        wc = (
            wfull[c * IP : (c + 1) * IP]
            .reshape(NB // 2, 2, 128, FW)
            .transpose(0, 2, 1, 3)
            .reshape(NB // 2, 128, 2 * FW)
        )
        xc = (
            xt[c * IP : (c + 1) * IP]
            .reshape(NB, 4, D, B)
            .transpose(1, 2, 0, 3)
            .reshape(128, NB, B)
        )
        in_maps.append(
            {
                "w_in": np.ascontiguousarray(wc),
                "x_in": np.ascontiguousarray(xc),
            }
        )
    return in_maps


def _assemble_out(results: list[dict]) -> np.ndarray:
    parts = []
    for c in range(NCORES):
        o = results[c]["out"]  # [128, NB, D] with p = i4*32 + b
        o = o.reshape(4, 32, NB, D).transpose(1, 2, 0, 3).reshape(B, IP, D)
        parts.append(o)
    return np.ascontiguousarray(np.concatenate(parts, axis=1))


def _run(x: np.ndarray, W: np.ndarray, **kwargs):
    nc = _get_nc()
    in_maps = _prep_inputs(x, W)
    res = run_bass_kernel_spmd(
        nc, in_maps, core_ids=list(range(NCORES)), **kwargs
    )
    return _assemble_out(res.results), res


def kernel(x: np.ndarray, W: np.ndarray) -> np.ndarray:
    out, _ = _run(x, W)
    return out
